# revision 32
# baseline (speedup 1.0000x reference)
"""ClusterLoss Trainium2 Bass kernel (8-core SPMD).

Problem: features [32768, 1024] f32, 2048 identities x 16 contiguous images.
Returns (cluster_loss scalar, intra_max_distance [2048], inter_min_distance [2048]).

Sharding: data-parallel over identities (256 ids / core). Each core:
  - computes its centers + intra-max distances locally (f32),
  - PE-transposes its centers, AllGathers an fp16 [1026, 256] payload
    (rows 0..1023 = centers^T, rows 1024/1025 = ||c||^2 split hi/lo),
  - computes w_ij = <c_i, c_j> - 0.5*c2_j for its 256 rows x all 2048 cols
    via fp16 PE matmuls accumulating in f32 PSUM (augmented contraction rows
    add the c2_j term), then v = -2*w + BIG*diag, row-min, + c2_i, sqrt.
Host only shards inputs, concatenates outputs, and computes the final
mean(relu(intra - inter + margin)) over the returned [2048] vectors.

Phase structure (critical path first): feature DMA -> center reduces ->
transposes -> AllGather -> rhs loads -> matmuls -> min/post. The intra
diff/square work overlaps the AllGather window. DMA issue is spread across
sequencers (SP: features, ACT: payload + outputs, PE: rhs) to avoid
head-of-line blocking on one in-order queue.
"""

import numpy as np

import concourse.bass as bass
import concourse.mybir as mybir
import concourse.tile as tile
from concourse import bacc, masks
from concourse.bass_utils import run_bass_kernel_spmd

# Problem constants (hardcoded per spec)
L = 2048          # identities
K = 16            # images per identity
D = 1024          # feature dim
N = L * K         # 32768 rows
NCORES = 8
IDS = L // NCORES  # 256 ids per core
G = 2              # id groups of 128 (partition dim)
H = 2              # halves of K (8 images each)
KH = K // H        # 8
MARGIN = 10.0
EPS = 1e-12
BIG = 57344.0      # >> any center distance^2; BIG/2 = 28672 exact in fp8e5

F32 = mybir.dt.float32
F16 = mybir.dt.float16
F8 = mybir.dt.float8e5
AX = mybir.AxisListType
ALU = mybir.AluOpType
ACTF = mybir.ActivationFunctionType

PAYROWS = D + 2    # 1026: centers^T rows + c2 hi row + c2 lo row


def _emit_rep(nc, tc, pools, io, rep, prev_dep, n_cores=NCORES, parts=("intra", "inter")):
    """Emit one full cluster-loss computation. Returns a tile whose write
    completes only after this rep's last result (used to chain reps when
    benchmarking).

    Two-stage AllGather pipeline: the payload for id-group g (128 ids) ships
    as soon as centers[g] is ready, so the g0 AllGather and the family-0
    matmuls hide under the g1 feature loads / second AllGather.
    """
    from concourse.tile_rust import add_dep_helper

    (fp, redp, maskp, rhsp, smallp, pp, dram) = pools
    (feat, fv, dbias, out_intra, out_inter, ident) = io

    centers = [
        pp.tile([128, D], F32, name=f"centers{g}_{rep}", tag=f"centers{g}")
        for g in range(G)
    ]
    redacc = [
        pp.tile([128, D], F32, name=f"redacc{g}_{rep}", tag=f"redacc{g}")
        for g in range(G)
    ]
    c2g = [
        pp.tile([128, 1], F32, name=f"c2g{g}_{rep}", tag=f"c2g{g}")
        for g in range(G)
    ]
    d2 = [
        pp.tile([128, K], F32, name=f"d2g{g}_{rep}", tag=f"d2g{g}")
        for g in range(G)
    ]
    # transposed centers: chunk dc occupies columns [dc*256, dc*256+256)
    # with the two g-halves inside
    lhsT_all = pp.tile([128, 8 * IDS], F16, name=f"lhsT_{rep}", tag="lhsT")

    # augmented contraction rows add -0.5*c2_j so the (-2) scale at the end
    # yields v = c2_j - 2*cc
    lhsT9 = pp.tile([128, 128], F16, name=f"lhsT9_{rep}", tag="lhsT9")
    nc.gpsimd.memset(lhsT9[:], 0.0)
    nc.gpsimd.memset(lhsT9[0:2, :], -0.5)

    c2hi_sb = pp.tile([1, IDS], F16, name=f"c2hi_sb_{rep}", tag="c2hi_sb")
    c2lo_sb = pp.tile([1, IDS], F16, name=f"c2lo_sb_{rep}", tag="c2lo_sb")

    dbias_sb = pp.tile([128, G * L], F8, name=f"dbias_sb_{rep}", tag="dbias_sb")
    for g in range(G):
        nc.sync.dma_start(out=dbias_sb[:, g * L:(g + 1) * L], in_=dbias[g])
    ident8 = pp.tile([128, 128], F8, name=f"ident8_{rep}", tag="ident8")
    nc.scalar.activation(out=ident8[:], in_=ident[:], func=ACTF.Copy)

    # per-family payload: rows 0..127 = that family's transposed-center
    # columns (8 chunks x 128 ids); rows 128/129 = c2 hi/lo
    pay = [
        dram.tile([130, 8 * 128], F16, name=f"pay{g}_{rep}", tag=f"pay{g}")
        for g in range(G)
    ]
    agout = [
        dram.tile([NCORES * 130, 8 * 128], F16, addr_space="Shared",
                  name=f"agout{g}_{rep}")
        for g in range(G)
    ]

    # ========== Phase A: loads, centers, per-family payload + AG ==========
    Q = 4  # quarter tiles of 4 images each
    KQ = K // Q
    ftiles = {}
    adds = {}
    for g in range(G):
        with tc.tile_pool(name=f"redtmp{rep}_{g}", bufs=1,
                          space="PSUM") as redtmp:
            for q in range(Q):
                ft = fp.tile([128, KQ * D], F32, name="ft", tag="ft")
                if prev_dep is not None:
                    # zero-valued WAW gate: forces this rep's loads after
                    # the previous rep's final result (serial benchmarking)
                    nc.vector.scalar_tensor_tensor(
                        out=ft[:, 0:1], in0=prev_dep[:], scalar=0.0,
                        in1=prev_dep[:], op0=ALU.mult, op1=ALU.mult,
                    )
                nc.sync.dma_start(out=ft[:], in_=fv[g, q])
                ftiles[(g, q)] = ft
                if q == 0:
                    nc.vector.tensor_reduce(
                        out=redacc[g][:],
                        in_=ft[:].rearrange("p (k d) -> p d k", k=KQ),
                        axis=AX.X,
                        op=ALU.add,
                    )
                else:
                    rt = redtmp.tile([128, D], F32, name="rt", tag="rt",
                                     space="PSUM", bufs=1)
                    nc.vector.tensor_reduce(
                        out=rt[:],
                        in_=ft[:].rearrange("p (k d) -> p d k", k=KQ),
                        axis=AX.X,
                        op=ALU.add,
                    )
                    a = nc.vector.tensor_tensor(
                        out=redacc[g][:], in0=redacc[g][:], in1=rt[:],
                        op=ALU.add,
                    )
                    adds[(g, q)] = a
            nc.scalar.activation(
                out=centers[g][:], in_=redacc[g][:], func=ACTF.Copy,
                scale=1.0 / K,
            )
            # c2 = ||center||^2 (scratch output to PSUM, value unused)
            scr2 = redtmp.tile([128, D], F32, name="scr", tag="scr",
                               space="PSUM", bufs=1)
            nc.scalar.activation(
                out=scr2[:], in_=centers[g][:], func=ACTF.Square,
                accum_out=c2g[g][:],
            )
        # c2 hi/lo fp16 split (column form)
        hi = smallp.tile([128, 1], F16, name="c2hic", tag="c2hic")
        nc.scalar.activation(out=hi[:], in_=c2g[g][:], func=ACTF.Copy)
        lo = smallp.tile([128, 1], F32, name="c2loc", tag="c2loc")
        nc.vector.tensor_tensor(
            out=lo[:], in0=c2g[g][:], in1=hi[:], op=ALU.subtract
        )
        hi32 = smallp.tile([128, 1], F32, name="c2hic32", tag="c2hic32")
        nc.scalar.activation(out=hi32[:], in_=hi[:], func=ACTF.Copy)

        # transpose centers -> lhsT_all columns for this family
        with tc.tile_pool(name=f"pst{rep}_{g}", bufs=4, space="PSUM") as pstp:
            for dc in range(8):
                ps = pstp.tile([128, 128], F32, name="ps", tag="ps",
                               space="PSUM")
                nc.tensor.transpose(
                    ps[:], centers[g][:, dc * 128:(dc + 1) * 128], ident[:]
                )
                dst = lhsT_all[:, dc * IDS + g * 128:
                               dc * IDS + (g + 1) * 128]
                nc.scalar.activation(out=dst, in_=ps[:], func=ACTF.Copy)
            psh = pstp.tile([1, 128], F32, name="psh", tag="ps", space="PSUM")
            nc.tensor.transpose(psh[:], hi32[:], ident[:])
            nc.scalar.activation(
                out=c2hi_sb[:, g * 128:(g + 1) * 128], in_=psh[:],
                func=ACTF.Copy,
            )
            psl = pstp.tile([1, 128], F32, name="psl", tag="ps", space="PSUM")
            nc.tensor.transpose(psl[:], lo[:], ident[:])
            nc.scalar.activation(
                out=c2lo_sb[:, g * 128:(g + 1) * 128], in_=psl[:],
                func=ACTF.Copy,
            )

        # family payload: lhsT columns of this g + its c2 rows, then AG
        nc.scalar.dma_start(
            out=pay[g][0:128, :].rearrange("r (dc j) -> r dc j", dc=8),
            in_=lhsT_all[:].rearrange("p (dc gg j) -> p dc gg j",
                                      dc=8, gg=G)[:, :, g],
        )
        nc.scalar.dma_start(
            out=pay[g][128:129, 0:128], in_=c2hi_sb[:, g * 128:(g + 1) * 128]
        )
        nc.scalar.dma_start(
            out=pay[g][129:130, 0:128], in_=c2lo_sb[:, g * 128:(g + 1) * 128]
        )
        if n_cores > 1:
            nc.gpsimd.collective_compute(
                "AllGather",
                ALU.bypass,
                replica_groups=[list(range(n_cores))],
                ins=[pay[g].opt()],
                outs=[agout[g].opt()],
            )
        else:
            # collective-free variant for cost-model timeline analysis
            nc.sync.dma_start(out=agout[g][0:130, :], in_=pay[g])

    # ========= Phase B: intra diff/square work (overlaps AllGathers) ======
    last_add = adds[(G - 1, Q - 1)]
    intra_last = None
    for g in list(range(G)) if "intra" in parts else []:
        for q in range(Q):
            ft = ftiles[(g, q)]
            ftv = ft[:].rearrange("p (k d) -> p k d", k=KQ)
            cb = centers[g][:][:, None, :].broadcast_to([128, KQ, D])
            di = nc.vector.tensor_tensor(
                out=ftv, in0=ftv, in1=cb, op=ALU.subtract
            )
            # keep the center-reduce chain ahead of diffs on DVE
            add_dep_helper(di.ins, last_add.ins, sync=False,
                           reason="diffs after center reduces")
            for k in range(KQ):
                col = q * KQ + k
                nc.scalar.activation(
                    out=ft[:, k * D:(k + 1) * D],
                    in_=ft[:, k * D:(k + 1) * D],
                    func=ACTF.Square,
                    accum_out=d2[g][:, col:col + 1],
                )
        dmax = smallp.tile([128, 1], F32, name="dmax", tag="dmax")
        nc.vector.tensor_reduce(
            out=dmax[:], in_=d2[g][:], axis=AX.X, op=ALU.max
        )
        nc.vector.tensor_scalar_max(dmax[:], dmax[:], EPS)
        intra_sb = smallp.tile([128, 1], F32, name="intra_sb", tag="intra_sb")
        nc.scalar.activation(out=intra_sb[:], in_=dmax[:], func=ACTF.Sqrt)
        nc.scalar.dma_start(out=out_intra[g], in_=intra_sb[:])
        intra_last = intra_sb

    if "inter" not in parts:
        return intra_last if intra_last is not None else c2g[G - 1]

    # ========= Phase C: per-family rhs readback, matmuls, max ============
    # agout[f] row c*130 + p, col dc*128 + jj  ==  C_c[f*128+jj, dc*128 + p]
    # vps[g] columns: [family0: 1024][family1: 1024]; diagonal of vps[g]
    # lives in family g at column g*1024 + c*128 + p (preloaded via dbias).
    minp = [
        smallp.tile([128, 4], F32, name=f"minp{g}_{rep}", tag=f"minp{g}")
        for g in range(G)
    ]
    rhs9 = [
        pp.tile([2, 8 * 128], F16, name=f"rhs9_{f}_{rep}", tag=f"rhs9_{f}")
        for f in range(G)
    ]
    vpp = [None, None]
    vps = [[None, None], [None, None]]  # [f][g]
    inter_done = None
    for f in range(G):
        agr = agout[f].rearrange("(c r) j -> r c j", c=NCORES)
        vpp[f] = tc.tile_pool(name=f"vp{rep}_{f}", bufs=1, space="PSUM")
        vpf = vpp[f].__enter__()
        for g in range(G):
            vps[f][g] = vpf.tile([128, 8 * 128], F32, name=f"vps{f}{g}",
                                 tag=f"vps{f}{g}")
            # preload the diagonal bias via identity-weighted fp8 matmul
            for nh in range(2):
                nc.tensor.matmul(
                    vps[f][g][:, nh * 512:(nh + 1) * 512],
                    lhsT=ident8[:],
                    rhs=dbias_sb[:, g * L + f * 1024 + nh * 512:
                                 g * L + f * 1024 + (nh + 1) * 512],
                    start=True,
                    stop=False,
                )
        for i in range(2):
            nc.gpsimd.dma_start(
                out=rhs9[f][i:i + 1, :].rearrange("p (c j) -> p c j",
                                                  c=NCORES),
                in_=agr[128 + i][:, None, 0:128],
            )
        rhs_f = rhsp.tile([128, 8 * 8 * 128], F16, name=f"rhs{f}",
                          tag=f"rhs{f}")
        for dc in range(8):
            nc.sync.dma_start(
                out=rhs_f[:, dc * 1024:(dc + 1) * 1024].rearrange(
                    "p (c j) -> p c j", c=NCORES
                ),
                in_=agr[0:128, :, dc * 128:(dc + 1) * 128],
            )
            for g in range(G):
                lt = lhsT_all[:, dc * IDS + g * 128: dc * IDS + (g + 1) * 128]
                for nh in range(2):
                    nc.tensor.matmul(
                        vps[f][g][:, nh * 512:(nh + 1) * 512],
                        lhsT=lt,
                        rhs=rhs_f[:, dc * 1024 + nh * 512:
                                  dc * 1024 + (nh + 1) * 512],
                        start=False,
                        stop=False,
                        skip_group_check=True,
                    )
        for g in range(G):
            for nh in range(2):
                nc.tensor.matmul(
                    vps[f][g][:, nh * 512:(nh + 1) * 512],
                    lhsT=lhsT9[0:2, :],
                    rhs=rhs9[f][:, nh * 512:(nh + 1) * 512],
                    start=False,
                    stop=True,
                    skip_group_check=True,
                )
            # row-max of w per 512 block; min_j v = -2 * max_j w
            for nh in range(2):
                nc.vector.tensor_reduce(
                    out=minp[g][:, f * 2 + nh:f * 2 + nh + 1],
                    in_=vps[f][g][:, nh * 512:(nh + 1) * 512],
                    axis=AX.X,
                    op=ALU.max,
                )

    for g in range(G):
        minv = smallp.tile([128, 1], F32, name="minv", tag="minv")
        nc.vector.tensor_reduce(
            out=minv[:], in_=minp[g][:], axis=AX.X, op=ALU.max
        )
        # inter^2 = c2_i - 2 * max_j w
        nc.vector.scalar_tensor_tensor(
            out=minv[:], in0=minv[:], scalar=-2.0, in1=c2g[g][:],
            op0=ALU.mult, op1=ALU.add,
        )
        nc.vector.tensor_scalar_max(minv[:], minv[:], EPS)
        inter_sb = smallp.tile([128, 1], F32, name="inter_sb", tag="inter_sb")
        nc.scalar.activation(out=inter_sb[:], in_=minv[:], func=ACTF.Sqrt)
        nc.scalar.dma_start(out=out_inter[g], in_=inter_sb[:])
        inter_done = inter_sb
    for f in reversed(range(G)):
        vpp[f].__exit__(None, None, None)
    return inter_done


def build_nc(reps=1, n_cores=NCORES, parts=('intra', 'inter')):
    nc = bacc.Bacc(
        "TRN2",
        target_bir_lowering=False,
        debug=False,
        num_devices=n_cores,
    )

    feat = nc.dram_tensor("features", [N // NCORES, D], F32, kind="ExternalInput")
    dbias = nc.dram_tensor("dbias", [G, 128, L], F8, kind="ExternalInput")
    out_intra = nc.dram_tensor("out_intra", [G, 128, 1], F32, kind="ExternalOutput")
    out_inter = nc.dram_tensor("out_inter", [G, 128, 1], F32, kind="ExternalOutput")

    # row = (g*128 + p)*16 + q*4 + k  ->  [g, q, p, (k d)]
    fv = feat.rearrange("(g p q k) d -> g q p (k d)", g=G, p=128, q=4, k=4)

    with tile.TileContext(nc) as tc:
        with (
            tc.tile_pool(name="fp", bufs=8) as fp,
            tc.tile_pool(name="redp", bufs=1) as redp,
            tc.tile_pool(name="maskp", bufs=1) as maskp,
            tc.tile_pool(name="rhsp", bufs=1) as rhsp,
            tc.tile_pool(name="smallp", bufs=2) as smallp,
            tc.tile_pool(name="persist", bufs=1) as pp,
            tc.tile_pool(name="dram", bufs=1, space="DRAM") as dram,
        ):
            ident = pp.tile([128, 128], F32, name="ident")
            masks.make_identity(nc, ident[:])

            pools = (fp, redp, maskp, rhsp, smallp, pp, dram)
            io = (feat, fv, dbias, out_intra, out_inter, ident)

            prev = None
            for rep in range(reps):
                prev = _emit_rep(nc, tc, pools, io, rep, prev, n_cores, parts)

    nc.compile()
    return nc


_CACHE = {}


def _get_nc(reps=1, n_cores=NCORES, parts=("intra", "inter")):
    key = f"nc{reps}_{n_cores}_{'_'.join(sorted(parts))}"
    if key not in _CACHE:
        _CACHE[key] = build_nc(reps, n_cores, parts)
    return _CACHE[key]


def make_in_maps(features: np.ndarray):
    features = np.asarray(features, dtype=np.float32)
    in_maps = []
    rows = N // NCORES
    for c in range(NCORES):
        sh = np.ascontiguousarray(features[c * rows:(c + 1) * rows])
        import ml_dtypes
        db = np.zeros((G, 128, L), ml_dtypes.float8_e5m2)
        for g in range(G):
            off = g * 1024 + c * 128
            db[g, np.arange(128), off + np.arange(128)] = -BIG / 2
        in_maps.append({"features": sh, "dbias": db})
    return in_maps


def kernel(features, targets=None, **unused):
    nc = _get_nc()
    in_maps = make_in_maps(features)
    res = run_bass_kernel_spmd(nc, in_maps, core_ids=list(range(NCORES)))
    intra = np.concatenate(
        [res.results[c]["out_intra"].reshape(IDS) for c in range(NCORES)]
    ).astype(np.float32)
    inter = np.concatenate(
        [res.results[c]["out_inter"].reshape(IDS) for c in range(NCORES)]
    ).astype(np.float32)
    loss = np.float32(
        np.mean(np.maximum(intra - inter + np.float32(MARGIN), np.float32(0.0)))
    )
    return loss, intra, inter


# revision 33
# speedup vs baseline: 47132.8388x; 47132.8388x over previous
"""ClusterLoss Trainium2 Bass kernel (8-core SPMD).

Problem: features [32768, 1024] f32, 2048 identities x 16 contiguous images.
Returns (cluster_loss scalar, intra_max_distance [2048], inter_min_distance [2048]).

Sharding: data-parallel over identities (256 ids / core). Each core:
  - computes its centers + intra-max distances locally (f32),
  - PE-transposes its centers, AllGathers an fp16 [1026, 256] payload
    (rows 0..1023 = centers^T, rows 1024/1025 = ||c||^2 split hi/lo),
  - computes w_ij = <c_i, c_j> - 0.5*c2_j for its 256 rows x all 2048 cols
    via fp16 PE matmuls accumulating in f32 PSUM (augmented contraction rows
    add the c2_j term), then v = -2*w + BIG*diag, row-min, + c2_i, sqrt.
Host only shards inputs, concatenates outputs, and computes the final
mean(relu(intra - inter + margin)) over the returned [2048] vectors.

Phase structure (critical path first): feature DMA -> center reduces ->
transposes -> AllGather -> rhs loads -> matmuls -> min/post. The intra
diff/square work overlaps the AllGather window. DMA issue is spread across
sequencers (SP: features, ACT: payload + outputs, PE: rhs) to avoid
head-of-line blocking on one in-order queue.
"""

import numpy as np

import concourse.bass as bass
import concourse.mybir as mybir
import concourse.tile as tile
from concourse import bacc, masks
from concourse.bass_utils import run_bass_kernel_spmd

# Problem constants (hardcoded per spec)
L = 2048          # identities
K = 16            # images per identity
D = 1024          # feature dim
N = L * K         # 32768 rows
NCORES = 8
IDS = L // NCORES  # 256 ids per core
G = 2              # id groups of 128 (partition dim)
H = 2              # halves of K (8 images each)
KH = K // H        # 8
MARGIN = 10.0
EPS = 1e-12
BIG = 57344.0      # >> any center distance^2; BIG/2 = 28672 exact in fp8e5

F32 = mybir.dt.float32
F16 = mybir.dt.float16
F8 = mybir.dt.float8e5
AX = mybir.AxisListType
ALU = mybir.AluOpType
ACTF = mybir.ActivationFunctionType

PAYROWS = D + 2    # 1026: centers^T rows + c2 hi row + c2 lo row


def _emit_rep(nc, tc, pools, io, rep, prev_dep, n_cores=NCORES, parts=("intra", "inter")):
    """Emit one full cluster-loss computation. Returns a tile whose write
    completes only after this rep's last result (used to chain reps when
    benchmarking).

    Two-stage AllGather pipeline: the payload for id-group g (128 ids) ships
    as soon as centers[g] is ready, so the g0 AllGather and the family-0
    matmuls hide under the g1 feature loads / second AllGather.
    """
    from concourse.tile_rust import add_dep_helper

    (fp, redp, maskp, rhsp, smallp, pp, dram) = pools
    (feat, fv, dbias, out_intra, out_inter, ident) = io

    centers = [
        pp.tile([128, D], F32, name=f"centers{g}_{rep}", tag=f"centers{g}")
        for g in range(G)
    ]
    redacc = [
        pp.tile([128, D], F32, name=f"redacc{g}_{rep}", tag=f"redacc{g}")
        for g in range(G)
    ]
    c2g = [
        pp.tile([128, 1], F32, name=f"c2g{g}_{rep}", tag=f"c2g{g}")
        for g in range(G)
    ]
    d2 = [
        pp.tile([128, K], F32, name=f"d2g{g}_{rep}", tag=f"d2g{g}")
        for g in range(G)
    ]
    # transposed centers: chunk dc occupies columns [dc*256, dc*256+256)
    # with the two g-halves inside
    lhsT_all = pp.tile([128, 8 * IDS], F16, name=f"lhsT_{rep}", tag="lhsT")

    # augmented contraction rows add -0.5*c2_j so the (-2) scale at the end
    # yields v = c2_j - 2*cc
    lhsT9 = pp.tile([128, 128], F16, name=f"lhsT9_{rep}", tag="lhsT9")
    nc.gpsimd.memset(lhsT9[:], 0.0)
    nc.gpsimd.memset(lhsT9[0:2, :], -0.5)

    c2hi_sb = pp.tile([1, IDS], F16, name=f"c2hi_sb_{rep}", tag="c2hi_sb")
    c2lo_sb = pp.tile([1, IDS], F16, name=f"c2lo_sb_{rep}", tag="c2lo_sb")

    dbias_sb = pp.tile([128, G * L], F8, name=f"dbias_sb_{rep}", tag="dbias_sb")
    for g in range(G):
        nc.sync.dma_start(out=dbias_sb[:, g * L:(g + 1) * L], in_=dbias[g])
    ident8 = pp.tile([128, 128], F8, name=f"ident8_{rep}", tag="ident8")
    nc.scalar.activation(out=ident8[:], in_=ident[:], func=ACTF.Copy)

    # per-family payload: rows 0..127 = that family's transposed-center
    # columns (8 chunks x 128 ids); rows 128/129 = c2 hi/lo
    pay = [
        dram.tile([130, 8 * 128], F16, name=f"pay{g}_{rep}", tag=f"pay{g}")
        for g in range(G)
    ]
    agout = [
        dram.tile([NCORES * 130, 8 * 128], F16, addr_space="Shared",
                  name=f"agout{g}_{rep}")
        for g in range(G)
    ]

    # ========== Phase A: loads, centers, per-family payload + AG ==========
    Q = 4  # quarter tiles of 4 images each
    KQ = K // Q
    ftiles = {}
    adds = {}
    for g in range(G):
        with tc.tile_pool(name=f"redtmp{rep}_{g}", bufs=1,
                          space="PSUM") as redtmp:
            for q in range(Q):
                ft = fp.tile([128, KQ * D], F32, name="ft", tag="ft")
                if prev_dep is not None:
                    # zero-valued WAW gate: forces this rep's loads after
                    # the previous rep's final result (serial benchmarking)
                    nc.vector.scalar_tensor_tensor(
                        out=ft[:, 0:1], in0=prev_dep[:], scalar=0.0,
                        in1=prev_dep[:], op0=ALU.mult, op1=ALU.mult,
                    )
                nc.sync.dma_start(out=ft[:], in_=fv[g, q])
                ftiles[(g, q)] = ft
                if q == 0:
                    nc.vector.tensor_reduce(
                        out=redacc[g][:],
                        in_=ft[:].rearrange("p (k d) -> p d k", k=KQ),
                        axis=AX.X,
                        op=ALU.add,
                    )
                else:
                    rt = redtmp.tile([128, D], F32, name="rt", tag="rt",
                                     space="PSUM", bufs=1)
                    nc.vector.tensor_reduce(
                        out=rt[:],
                        in_=ft[:].rearrange("p (k d) -> p d k", k=KQ),
                        axis=AX.X,
                        op=ALU.add,
                    )
                    a = nc.vector.tensor_tensor(
                        out=redacc[g][:], in0=redacc[g][:], in1=rt[:],
                        op=ALU.add,
                    )
                    adds[(g, q)] = a
            nc.scalar.activation(
                out=centers[g][:], in_=redacc[g][:], func=ACTF.Copy,
                scale=1.0 / K,
            )
            # c2 = ||center||^2 (scratch output to PSUM, value unused)
            scr2 = redtmp.tile([128, D], F32, name="scr", tag="rt",
                               space="PSUM", bufs=1)
            nc.scalar.activation(
                out=scr2[:], in_=centers[g][:], func=ACTF.Square,
                accum_out=c2g[g][:],
            )
        # c2 hi/lo fp16 split (column form)
        hi = smallp.tile([128, 1], F16, name="c2hic", tag="c2hic")
        nc.scalar.activation(out=hi[:], in_=c2g[g][:], func=ACTF.Copy)
        lo = smallp.tile([128, 1], F32, name="c2loc", tag="c2loc")
        nc.vector.tensor_tensor(
            out=lo[:], in0=c2g[g][:], in1=hi[:], op=ALU.subtract
        )
        hi32 = smallp.tile([128, 1], F32, name="c2hic32", tag="c2hic32")
        nc.scalar.activation(out=hi32[:], in_=hi[:], func=ACTF.Copy)

        # transpose centers -> lhsT_all columns for this family
        with tc.tile_pool(name=f"pst{rep}_{g}", bufs=2, space="PSUM") as pstp:
            for dc in range(8):
                ps = pstp.tile([128, 128], F32, name="ps", tag="ps",
                               space="PSUM")
                nc.tensor.transpose(
                    ps[:], centers[g][:, dc * 128:(dc + 1) * 128], ident[:]
                )
                dst = lhsT_all[:, dc * IDS + g * 128:
                               dc * IDS + (g + 1) * 128]
                nc.scalar.activation(out=dst, in_=ps[:], func=ACTF.Copy)
            psh = pstp.tile([1, 128], F32, name="psh", tag="ps", space="PSUM")
            nc.tensor.transpose(psh[:], hi32[:], ident[:])
            nc.scalar.activation(
                out=c2hi_sb[:, g * 128:(g + 1) * 128], in_=psh[:],
                func=ACTF.Copy,
            )
            psl = pstp.tile([1, 128], F32, name="psl", tag="ps", space="PSUM")
            nc.tensor.transpose(psl[:], lo[:], ident[:])
            nc.scalar.activation(
                out=c2lo_sb[:, g * 128:(g + 1) * 128], in_=psl[:],
                func=ACTF.Copy,
            )

        # family payload: lhsT columns of this g + its c2 rows, then AG
        nc.scalar.dma_start(
            out=pay[g][0:128, :].rearrange("r (dc j) -> r dc j", dc=8),
            in_=lhsT_all[:].rearrange("p (dc gg j) -> p dc gg j",
                                      dc=8, gg=G)[:, :, g],
        )
        nc.scalar.dma_start(
            out=pay[g][128:129, 0:128], in_=c2hi_sb[:, g * 128:(g + 1) * 128]
        )
        nc.scalar.dma_start(
            out=pay[g][129:130, 0:128], in_=c2lo_sb[:, g * 128:(g + 1) * 128]
        )
        if n_cores > 1:
            nc.gpsimd.collective_compute(
                "AllGather",
                ALU.bypass,
                replica_groups=[list(range(n_cores))],
                ins=[pay[g].opt()],
                outs=[agout[g].opt()],
            )
        else:
            # collective-free variant for cost-model timeline analysis
            nc.sync.dma_start(out=agout[g][0:130, :], in_=pay[g])

    # ========= Phase B: intra diff/square work (overlaps AllGathers) ======
    last_add = adds[(G - 1, Q - 1)]
    intra_last = None
    for g in list(range(G)) if "intra" in parts else []:
        for q in range(Q):
            ft = ftiles[(g, q)]
            ftv = ft[:].rearrange("p (k d) -> p k d", k=KQ)
            cb = centers[g][:][:, None, :].broadcast_to([128, KQ, D])
            di = nc.vector.tensor_tensor(
                out=ftv, in0=ftv, in1=cb, op=ALU.subtract
            )
            # keep the center-reduce chain ahead of diffs on DVE
            add_dep_helper(di.ins, last_add.ins, sync=False,
                           reason="diffs after center reduces")
            for k in range(KQ):
                col = q * KQ + k
                nc.scalar.activation(
                    out=ft[:, k * D:(k + 1) * D],
                    in_=ft[:, k * D:(k + 1) * D],
                    func=ACTF.Square,
                    accum_out=d2[g][:, col:col + 1],
                )
        dmax = smallp.tile([128, 1], F32, name="dmax", tag="dmax")
        nc.vector.tensor_reduce(
            out=dmax[:], in_=d2[g][:], axis=AX.X, op=ALU.max
        )
        nc.vector.tensor_scalar_max(dmax[:], dmax[:], EPS)
        intra_sb = smallp.tile([128, 1], F32, name="intra_sb", tag="intra_sb")
        nc.scalar.activation(out=intra_sb[:], in_=dmax[:], func=ACTF.Sqrt)
        nc.scalar.dma_start(out=out_intra[g], in_=intra_sb[:])
        intra_last = intra_sb

    if "inter" not in parts:
        return intra_last if intra_last is not None else c2g[G - 1]

    # ========= Phase C: per-family rhs readback, matmuls, max ============
    # agout[f] row c*130 + p, col dc*128 + jj  ==  C_c[f*128+jj, dc*128 + p]
    # vps[g] columns: [family0: 1024][family1: 1024]; diagonal of vps[g]
    # lives in family g at column g*1024 + c*128 + p (preloaded via dbias).
    minp = [
        smallp.tile([128, 4], F32, name=f"minp{g}_{rep}", tag=f"minp{g}")
        for g in range(G)
    ]
    rhs9 = [
        pp.tile([2, 8 * 128], F16, name=f"rhs9_{f}_{rep}", tag=f"rhs9_{f}")
        for f in range(G)
    ]
    vpp = [None, None]
    vps = [[None, None], [None, None]]  # [f][g]
    inter_done = None
    for f in range(G):
        agr = agout[f].rearrange("(c r) j -> r c j", c=NCORES)
        vpp[f] = tc.tile_pool(name=f"vp{rep}_{f}", bufs=1, space="PSUM")
        vpf = vpp[f].__enter__()
        for g in range(G):
            vps[f][g] = vpf.tile([128, 8 * 128], F32, name=f"vps{f}{g}",
                                 tag=f"vps{f}{g}")
            # preload the diagonal bias via identity-weighted fp8 matmul
            for nh in range(2):
                nc.tensor.matmul(
                    vps[f][g][:, nh * 512:(nh + 1) * 512],
                    lhsT=ident8[:],
                    rhs=dbias_sb[:, g * L + f * 1024 + nh * 512:
                                 g * L + f * 1024 + (nh + 1) * 512],
                    start=True,
                    stop=False,
                )
        for i in range(2):
            nc.sync.dma_start(
                out=rhs9[f][i:i + 1, :].rearrange("p (c j) -> p c j",
                                                  c=NCORES),
                in_=agr[128 + i][:, None, 0:128],
            )
        rhs_f = rhsp.tile([128, 8 * 8 * 128], F16, name=f"rhs{f}",
                          tag=f"rhs{f}")
        for dc in range(8):
            nc.sync.dma_start(
                out=rhs_f[:, dc * 1024:(dc + 1) * 1024].rearrange(
                    "p (c j) -> p c j", c=NCORES
                ),
                in_=agr[0:128, :, dc * 128:(dc + 1) * 128],
            )
            for g in range(G):
                lt = lhsT_all[:, dc * IDS + g * 128: dc * IDS + (g + 1) * 128]
                for nh in range(2):
                    nc.tensor.matmul(
                        vps[f][g][:, nh * 512:(nh + 1) * 512],
                        lhsT=lt,
                        rhs=rhs_f[:, dc * 1024 + nh * 512:
                                  dc * 1024 + (nh + 1) * 512],
                        start=False,
                        stop=False,
                        skip_group_check=True,
                    )
        for g in range(G):
            for nh in range(2):
                nc.tensor.matmul(
                    vps[f][g][:, nh * 512:(nh + 1) * 512],
                    lhsT=lhsT9[0:2, :],
                    rhs=rhs9[f][:, nh * 512:(nh + 1) * 512],
                    start=False,
                    stop=True,
                    skip_group_check=True,
                )
            # row-max of w per 512 block; min_j v = -2 * max_j w
            for nh in range(2):
                nc.vector.tensor_reduce(
                    out=minp[g][:, f * 2 + nh:f * 2 + nh + 1],
                    in_=vps[f][g][:, nh * 512:(nh + 1) * 512],
                    axis=AX.X,
                    op=ALU.max,
                )

    for g in range(G):
        minv = smallp.tile([128, 1], F32, name="minv", tag="minv")
        nc.vector.tensor_reduce(
            out=minv[:], in_=minp[g][:], axis=AX.X, op=ALU.max
        )
        # inter^2 = c2_i - 2 * max_j w
        nc.vector.scalar_tensor_tensor(
            out=minv[:], in0=minv[:], scalar=-2.0, in1=c2g[g][:],
            op0=ALU.mult, op1=ALU.add,
        )
        nc.vector.tensor_scalar_max(minv[:], minv[:], EPS)
        inter_sb = smallp.tile([128, 1], F32, name="inter_sb", tag="inter_sb")
        nc.scalar.activation(out=inter_sb[:], in_=minv[:], func=ACTF.Sqrt)
        nc.scalar.dma_start(out=out_inter[g], in_=inter_sb[:])
        inter_done = inter_sb
    for f in reversed(range(G)):
        vpp[f].__exit__(None, None, None)
    return inter_done


def build_nc(reps=1, n_cores=NCORES, parts=('intra', 'inter')):
    nc = bacc.Bacc(
        "TRN2",
        target_bir_lowering=False,
        debug=False,
        num_devices=n_cores,
    )

    feat = nc.dram_tensor("features", [N // NCORES, D], F32, kind="ExternalInput")
    dbias = nc.dram_tensor("dbias", [G, 128, L], F8, kind="ExternalInput")
    out_intra = nc.dram_tensor("out_intra", [G, 128, 1], F32, kind="ExternalOutput")
    out_inter = nc.dram_tensor("out_inter", [G, 128, 1], F32, kind="ExternalOutput")

    # row = (g*128 + p)*16 + q*4 + k  ->  [g, q, p, (k d)]
    fv = feat.rearrange("(g p q k) d -> g q p (k d)", g=G, p=128, q=4, k=4)

    with tile.TileContext(nc) as tc:
        with (
            tc.tile_pool(name="fp", bufs=8) as fp,
            tc.tile_pool(name="redp", bufs=1) as redp,
            tc.tile_pool(name="maskp", bufs=1) as maskp,
            tc.tile_pool(name="rhsp", bufs=1) as rhsp,
            tc.tile_pool(name="smallp", bufs=2) as smallp,
            tc.tile_pool(name="persist", bufs=1) as pp,
            tc.tile_pool(name="dram", bufs=1, space="DRAM") as dram,
        ):
            ident = pp.tile([128, 128], F32, name="ident")
            masks.make_identity(nc, ident[:])

            pools = (fp, redp, maskp, rhsp, smallp, pp, dram)
            io = (feat, fv, dbias, out_intra, out_inter, ident)

            prev = None
            for rep in range(reps):
                prev = _emit_rep(nc, tc, pools, io, rep, prev, n_cores, parts)

    nc.compile()
    return nc


_CACHE = {}


def _get_nc(reps=1, n_cores=NCORES, parts=("intra", "inter")):
    key = f"nc{reps}_{n_cores}_{'_'.join(sorted(parts))}"
    if key not in _CACHE:
        _CACHE[key] = build_nc(reps, n_cores, parts)
    return _CACHE[key]


def make_in_maps(features: np.ndarray):
    features = np.asarray(features, dtype=np.float32)
    in_maps = []
    rows = N // NCORES
    for c in range(NCORES):
        sh = np.ascontiguousarray(features[c * rows:(c + 1) * rows])
        import ml_dtypes
        db = np.zeros((G, 128, L), ml_dtypes.float8_e5m2)
        for g in range(G):
            off = g * 1024 + c * 128
            db[g, np.arange(128), off + np.arange(128)] = -BIG / 2
        in_maps.append({"features": sh, "dbias": db})
    return in_maps


def kernel(features, targets=None, **unused):
    nc = _get_nc()
    in_maps = make_in_maps(features)
    res = run_bass_kernel_spmd(nc, in_maps, core_ids=list(range(NCORES)))
    intra = np.concatenate(
        [res.results[c]["out_intra"].reshape(IDS) for c in range(NCORES)]
    ).astype(np.float32)
    inter = np.concatenate(
        [res.results[c]["out_inter"].reshape(IDS) for c in range(NCORES)]
    ).astype(np.float32)
    loss = np.float32(
        np.mean(np.maximum(intra - inter + np.float32(MARGIN), np.float32(0.0)))
    )
    return loss, intra, inter
